# revision 1
# baseline (speedup 1.0000x reference)
"""MoDL (CNN denoiser + CG data-consistency MRI recon) on 8 Trainium2 NeuronCores.

Data-parallel over batch B=8, one image per core; batch coupling through the
CG scalars is preserved with one 16-byte AllReduce per iteration.

CNN (bf16, entirely in SBUF; ~230MB of baseline DRAM traffic -> ~5MB):
- Activations stored even rows on partitions 0-63, odd rows on 64-127,
  129 slots x 258 cols with baked-in zero pad; each 3x3 conv layer is 6
  accumulating matmuls of [128,128] bf16 stationary per output row-pair.
- Matmuls cover two row-pairs each (N=512) and rotate weights across 4 PSUM
  tiles so LDWEIGHTS amortizes; drains (bias+relu+bf16) split scalar/vector.
- Layer-1 im2col input borrows actb's partitions 0-11; layer-5 output goes
  through a small DRAM roundtrip into the CG layout.

CG (AtA = sum_c conj(C_c) . F^-1 M F (C_c . p) + lam p, 11 iterations):
- Fields interleaved [128, 1024] (free = t*512 + pl*256 + w), all DFT
  stages as data-stationary matmuls with concatenated bf16 F matrices
  [Fa|Fb] moving (8 x MM(512) per stage), so every stage's output lands
  pre-transposed and [re|im]-packed for the next stage.
- Coils processed in software-pipelined PAIRS so each stage's PSUM drain
  hides under the other coil's matmuls (PE FIFO never head-of-line blocks).
- All intermediates bf16 (fp32 PSUM accumulation); products use a
  host-prepared csm_swap = [-b|a] so each product slot is one TT op;
  coil sums via contiguous bf16 tree adds.
- One AllReduce of 4 scalars (pAp, rAp, ApAp, rr) per iteration;
  partition sums via a ones-vector matmul; alpha/beta from a one-step
  recurrence (rTrNew = rr - 2a*rAp + a^2*ApAp) with rr measured fresh each
  iteration so recurrence error does not compound.
- Engines: gpsimd does coil forming (SBUF-only; it cannot touch PSUM),
  vector does mask/products/trees/updates/dots, scalar does PSUM casts.

Note: vector.tensor_tensor_reduce crashes the device at runtime (compiles
fine) - use_ttr stays False.
"""

from contextlib import ExitStack

import numpy as np
import ml_dtypes

import concourse.bass as bass
import concourse.tile as tile
from concourse import mybir
from concourse.bass_utils import run_bass_kernel_spmd

FP = mybir.dt.float32
FPR = mybir.dt.float32r
BF = mybir.dt.bfloat16
AX = mybir.AxisListType
OP = mybir.AluOpType
AF = mybir.ActivationFunctionType

B, NCOIL, H, W = 8, 12, 256, 256
N_CG = 11
N_CORES = 8
HW = H * W
NSLOT = 129          # act rows slots (incl zero edge slot)
SW = 258             # act slot width (256 + 2 zero pad cols)
ACTF = NSLOT * SW    # act tile free size


# ---------------------------------------------------------------- host prep

def _bf(a):
    return np.asarray(a, np.float32).astype(ml_dtypes.bfloat16)


def _tall(plane):
    return np.ascontiguousarray(
        plane.reshape(2, 128, 256).transpose(1, 0, 2).reshape(128, 512))


def _interleave(re, im):
    rt, it = _tall(re), _tall(im)
    out = np.empty((128, 1024), np.float32)
    for t in range(2):
        out[:, t * 512:t * 512 + 256] = rt[:, t * 256:(t + 1) * 256]
        out[:, t * 512 + 256:t * 512 + 512] = it[:, t * 256:(t + 1) * 256]
    return out


def _prep_wmid(wl):
    # wg[6, 128, 128]; g = pair*3+dxi, rows h*64+ci, cols j*64+co
    # (odd-row output block always at column 64 so PSUM partition bases
    #  stay 64-aligned for the drain engines)
    cout, cin = wl.shape[0], wl.shape[1]
    wg = np.zeros((6, 128, 64 + cout), np.float32)
    for pair in range(2):
        for dxi in range(3):
            g = pair * 3 + dxi
            for h in range(2):
                for j in range(2):
                    dy = (1 - j if h == 0 else -j) if pair == 0 else \
                         (3 - j if h == 0 else 2 - j)
                    if 0 <= dy <= 2:
                        wg[g, h * 64:h * 64 + cin,
                           j * 64:j * 64 + cout] = wl[:, :, dy, dxi].T
    return wg


def _prep_wl1(w1):
    wl1 = np.zeros((2, 12, 128), np.float32)
    for pair in range(2):
        for dxi in range(3):
            for h in range(2):
                for ci in range(2):
                    p = dxi * 4 + h * 2 + ci
                    for j in range(2):
                        dy = (1 - j if h == 0 else -j) if pair == 0 else \
                             (3 - j if h == 0 else 2 - j)
                        if 0 <= dy <= 2:
                            wl1[pair, p, j * 64:j * 64 + 64] = w1[:, ci, dy, dxi]
    return wl1


def _prep_im1(atb_b):
    pad = np.zeros((2, 260, 260), np.float32)
    pad[:, 2:258, 2:258] = atb_b
    im1 = np.zeros((12, NSLOT, SW), np.float32)
    for dxi in range(3):
        for h in range(2):
            for ci in range(2):
                p = dxi * 4 + h * 2 + ci
                rows = (2 * np.arange(NSLOT) if h == 0
                        else 2 * np.arange(NSLOT) - 1)
                ok = (rows >= 0) & (rows < 256)
                im1[p, ok, :] = pad[ci, 2 + rows[ok], dxi:dxi + SW]
    im1[:, :, 0] = 0.0
    im1[:, :, 257] = 0.0
    return im1.reshape(12, ACTF)


def _make_fmcat():
    n = np.arange(256)
    Fm = np.exp(-2j * np.pi * np.outer(n, n) / 256) / 16.0
    FrT, FiT = _tall(Fm.real.astype(np.float32)), _tall(Fm.imag.astype(np.float32))
    fm = np.zeros((8, 128, 512), np.float32)
    for t in range(2):
        fr = FrT[:, t * 256:(t + 1) * 256]
        fi = FiT[:, t * 256:(t + 1) * 256]
        fm[0 + t] = np.concatenate([fr, fi], 1)        # FWD pl0
        fm[2 + t] = np.concatenate([-fi, fr], 1)       # FWD pl1
        fm[4 + t] = np.concatenate([fr, -fi], 1)       # INV pl0
        fm[6 + t] = np.concatenate([fi, fr], 1)        # INV pl1
    return fm


def _prep_shared(w1, b1, w2, b2, w3, b3, w4, b4, w5, b5, lam):
    out = {}
    out["wl1"] = _bf(_prep_wl1(np.asarray(w1, np.float32)))
    out["wg2"] = _bf(_prep_wmid(np.asarray(w2, np.float32)))
    out["wg3"] = _bf(_prep_wmid(np.asarray(w3, np.float32)))
    out["wg4"] = _bf(_prep_wmid(np.asarray(w4, np.float32)))
    out["wg5"] = _bf(_prep_wmid(np.asarray(w5, np.float32)))
    for i, bl in enumerate((b1, b2, b3, b4)):
        bl = np.asarray(bl, np.float32)
        out[f"bias{i + 1}"] = np.concatenate([bl, bl]).reshape(128, 1)
    b5 = np.asarray(b5, np.float32)
    bias5 = np.zeros((66, 1), np.float32)
    bias5[0:2, 0] = b5
    bias5[64:66, 0] = b5
    out["bias5"] = bias5
    out["fmcat"] = _bf(_make_fmcat())
    out["lam"] = np.asarray(lam, np.float32).reshape(1, 1)
    return out


def _prep_core(atb_b, csm_re_b, csm_im_b, mask_b):
    out = {}
    out["im1"] = _bf(_prep_im1(atb_b))
    out["atbint"] = _interleave(atb_b[0], atb_b[1])
    out["csmint"] = _bf(np.stack(
        [_interleave(csm_re_b[c], csm_im_b[c]) for c in range(NCOIL)]))
    # csmswap = [-b | a]: one-op prodB slots [-b*wre | a*wim]
    out["csmswap"] = _bf(np.stack(
        [_interleave(-csm_im_b[c], csm_re_b[c]) for c in range(NCOIL)]))
    out["mask2"] = _interleave(mask_b, mask_b)
    return out


# ------------------------------------------------------------- bass program

def build_nc(n_cg=N_CG, n_coil=NCOIL, n_cores=N_CORES, cnn=True, evsem=True,
             use_cc=True, gp_form=True, use_ttr=False):
    _uid = [0]

    def T(pool, shape, tag, dt=FP):
        _uid[0] += 1
        return pool.tile(shape, dt, tag=tag, name=f"{tag}_{_uid[0]}")

    nc = bass.Bass(num_devices=n_cores)
    group = [list(range(n_cores))]

    din = {}
    for name, shape, dt in [
        ("im1", [12, ACTF], BF), ("wl1", [2, 12, 128], BF),
        ("wg2", [6, 128, 128], BF), ("wg3", [6, 128, 128], BF),
        ("wg4", [6, 128, 128], BF), ("wg5", [6, 128, 66], BF),
        ("bias1", [128, 1], FP), ("bias2", [128, 1], FP),
        ("bias3", [128, 1], FP), ("bias4", [128, 1], FP), ("bias5", [66, 1], FP),
        ("fmcat", [8, 128, 512], BF),
        ("atbint", [128, 1024], FP),
        ("csmint", [n_coil, 128, 1024], BF),
        ("csmswap", [n_coil, 128, 1024], BF),
        ("mask2", [128, 1024], FP),
        ("lam", [1, 1], FP),
    ]:
        din[name] = nc.declare_dram_parameter(name, shape, dt, isOutput=False)
    dout = nc.declare_dram_parameter("out", [2, HW], FP, isOutput=True)

    hd5 = nc.dram_tensor("hd5", [2, 256, 256], BF)
    ccin = [nc.dram_tensor(f"ccin{i}", [1, 4], FP) for i in range(n_cg)]
    cc_space = "Shared" if n_cores > 4 else "Local"
    ccout = [nc.dram_tensor(f"ccout{i}", [1, 4], FP, addr_space=cc_space)
             for i in range(n_cg)]

    with tile.TileContext(nc) as tc, ExitStack() as ctx:  # noqa: SIM117
        consts = ctx.enter_context(tc.tile_pool(name="consts", bufs=1))
        lam128 = T(consts, [128, 1], "lam128")
        nc.sync.dma_start(out=lam128, in_=din["lam"][:].to_broadcast([128, 1]))
        lam1p = T(consts, [128, 1], "lam1p")
        nc.vector.tensor_scalar_add(lam1p, lam128, 1.0)
        ones128 = T(consts, [128, 1], "ones128")
        nc.vector.memset(ones128, 1.0)

        # CG const tiles up front; their DMAs are issued after the CNN
        # input DMAs (below) so layer 1 is not queued behind ~7MB of csm.
        cgc = ctx.enter_context(tc.tile_pool(name="cgc", bufs=1))
        fm = [T(cgc, [128, 512], f"fm{j}", BF) for j in range(8)]
        csm = [T(cgc, [128, 1024], f"csm{c}", BF) for c in range(n_coil)]
        csw = [T(cgc, [128, 1024], f"csw{c}", BF) for c in range(n_coil)]
        mask2 = T(cgc, [128, 1024], "mask2")

        def load_cg_consts():
            for j in range(8):
                nc.sync.dma_start(out=fm[j], in_=din["fmcat"][j])
            for c in range(n_coil):
                nc.sync.dma_start(out=csm[c], in_=din["csmint"][c])
                nc.sync.dma_start(out=csw[c], in_=din["csmswap"][c])
            nc.sync.dma_start(out=mask2, in_=din["mask2"][:])

        # =========================== CNN ===========================
        with tc.tile_pool(name="cnnw", bufs=1) as cw, \
             tc.tile_pool(name="cnnps", bufs=8, space="PSUM") as cps:
            acta = T(cw, [128, ACTF], "acta", BF)
            actb = T(cw, [128, ACTF], "actb", BF)
            for act in (acta, actb):
                av = act.rearrange("p (s c) -> p s c", c=SW)
                nc.vector.memset(av[:, :, 0:1], 0.0)       # left pad col
                nc.vector.memset(av[:, :, 257:258], 0.0)   # right pad col
                nc.vector.memset(av[0:64, 128:129, :], 0.0)   # even edge slot
                nc.vector.memset(av[64:128, 0:1, :], 0.0)     # odd edge slot
            wl1 = [T(cw, [12, 128], f"wl1_{pr}", BF) for pr in range(2)]
            for pr in range(2):
                nc.sync.dma_start(out=wl1[pr], in_=din["wl1"][pr])
            wg = {}
            for l in (2, 3, 4, 5):
                cols = 66 if l == 5 else 128
                wg[l] = [T(cw, [128, cols], f"w{l}g{g}", BF) for g in range(6)]
                for g in range(6):
                    nc.sync.dma_start(out=wg[l][g], in_=din[f"wg{l}"][g])
            bias = {}
            for l in (1, 2, 3, 4):
                bias[l] = T(cw, [128, 1], f"b{l}")
                nc.sync.dma_start(out=bias[l], in_=din[f"bias{l}"][:])
            bias[5] = T(cw, [66, 1], "b5")
            nc.sync.dma_start(out=bias[5], in_=din["bias5"][:])

            weng = [nc.scalar, nc.vector]  # gpsimd cannot read PSUM

            def writer(idx, dst_ap, src_ap, bias_ap, relu):
                e = weng[idx % 2]
                if e is nc.scalar:
                    e.activation(dst_ap, src_ap,
                                 AF.Relu if relu else AF.Identity, bias=bias_ap)
                elif relu:
                    e.tensor_scalar(dst_ap, src_ap, bias_ap, 0.0,
                                    op0=OP.add, op1=OP.max)
                else:
                    e.tensor_scalar(dst_ap, src_ap, bias_ap, None, op0=OP.add)

            def drain2(m, ps, dstv, bias_t, relu, np_half, widx):
                # ps [128, 512] covers rowpairs m, m+1. even half -> slots
                # (m, m+1); odd half -> slots (m+1, m+2).
                nh = np_half
                psv = ps.rearrange("p (s w) -> p s w", s=2)
                writer(widx, dstv[0:nh, m:m + 2, 1:257],
                       psv[0:nh], bias_t[0:nh, :], relu)
                writer(widx + 1, dstv[64:64 + nh, m + 1:m + 3, 1:257],
                       psv[64:64 + nh], bias_t[64:64 + nh, :], relu)

            if cnn:
                # ---- layer 1 (im1 in a sub-pool freed right after) ----
                actav_ = acta.rearrange("p (s c) -> p s c", c=SW)
                actbv_ = actb.rearrange("p (s c) -> p s c", c=SW)
                # layer-1 im2col input borrows actb's space (partitions
                # 0..11); its data cols are overwritten by L2's writers and
                # its edge slot is re-zeroed below.
                # chunked so layer 1 starts after the first slots land
                im1d = din["im1"][:].rearrange("p (s c) -> p s c", c=SW)
                for s0 in (0, 33, 66, 99):
                    s1_ = min(s0 + 33, NSLOT)
                    nc.sync.dma_start(
                        out=actbv_[0:12, s0:s1_, :], in_=im1d[:, s0:s1_, :])
                load_cg_consts()
                im1v = actbv_[0:12]
                for m in range(0, 128, 2):
                    ps = T(cps, [128, 512], "cps")
                    nc.tensor.matmul(ps, wl1[0], im1v[:, m:m + 2, 1:257],
                                     start=True, stop=False)
                    nc.tensor.matmul(ps, wl1[1], im1v[:, m + 1:m + 3, 1:257],
                                     start=False, stop=True)
                    drain2(m, ps, actav_, bias[1], True, 64, m)
                nc.vector.memset(actb[0:12, 128 * SW:129 * SW], 0.0)

                # ---- layers 2..5: 4 psum tiles per weight switch so
                # LDWEIGHTS amortizes over 4 matmuls ----
                for l, (srcv, dstv) in zip((2, 3, 4, 5),
                                           [(actav_, actbv_), (actbv_, actav_),
                                            (actav_, actbv_),
                                            (actbv_, actav_)]):
                    ncols = 66 if l == 5 else 128
                    nh = 2 if l == 5 else 64
                    for m0 in range(0, 128, 8):
                        pss = [T(cps, [128, 512], "cps") for _ in range(4)]
                        for pair in range(2):
                            for dxi in range(3):
                                g = pair * 3 + dxi
                                for k in range(4):
                                    slot = m0 + 2 * k + pair
                                    mv = srcv[:, slot:slot + 2, dxi:dxi + 256]
                                    nc.tensor.matmul(
                                        pss[k][0:ncols, :], wg[l][g], mv,
                                        start=(g == 0), stop=(g == 5))
                        for k in range(4):
                            drain2(m0 + 2 * k, pss[k], dstv, bias[l],
                                   l != 5, nh, m0 + 2 * k)

                # h5 act-layout -> DRAM natural [2,256,256] (bf16)
                actav = acta.rearrange("p (s c) -> p s c", c=SW)
                for co in range(2):
                    for h in range(2):
                        dst = hd5[co].rearrange("(r q) x -> q r x", q=2)[h:h + 1]
                        p0 = h * 64 + co
                        nc.sync.dma_start(
                            out=dst, in_=actav[p0:p0 + 1, h:h + 128, 1:257])

        # =========================== CG ===========================
        st = ctx.enter_context(tc.tile_pool(name="cgstate", bufs=1))
        wk = ctx.enter_context(tc.tile_pool(name="cgwork", bufs=2))
        wq = ctx.enter_context(tc.tile_pool(name="cgwq", bufs=1))
        sc = ctx.enter_context(tc.tile_pool(name="cgsmall", bufs=2))
        pp = ctx.enter_context(tc.tile_pool(name="cgps", bufs=8, space="PSUM"))
        prodp = ctx.enter_context(tc.tile_pool(name="cgprod", bufs=1))

        p_t = T(st, [128, 1024], "p")
        p_b = T(st, [128, 1024], "pb", BF)
        r_t = T(st, [128, 1024], "r")
        x_t = T(st, [128, 1024], "x")
        ap_t = T(st, [128, 1024], "ap")
        accs = [[T(st, [128, 512], f"acc{g}{pl}") for pl in range(2)]
                for g in range(3)]
        dotp = T(st, [128, 4], "dotp")
        nc.vector.memset(dotp, 0.0)

        def pview(tile_, pl):
            # [128, 2, 256] view of plane pl of an interleaved tile
            return tile_.rearrange("p (t q w) -> p q t w", t=2, q=2)[:, pl]

        # ---- rhs = (1+lam)*atb + lam*h5 ; r = p = rhs ; x = 0 ----
        with tc.tile_pool(name="cginit", bufs=1) as ip:
            atbp = T(ip, [128, 1024], "atbld")
            nc.sync.dma_start(out=atbp, in_=din["atbint"][:])
            nc.vector.tensor_scalar(r_t, atbp, lam1p[:], None, op0=OP.mult)
            if cnn:
                for pl in range(2):
                    h5b = T(ip, [128, 512], f"h5b{pl}", BF)
                    nc.sync.dma_start(
                        out=h5b.rearrange("p (t w) -> p t w", t=2),
                        in_=hd5[pl].rearrange("(t p) w -> p t w", p=128))
                    h5f = T(ip, [128, 512], f"h5f{pl}")
                    nc.scalar.copy(h5f, h5b)
                    nc.vector.scalar_tensor_tensor(
                        pview(r_t, pl), h5f.rearrange("p (t w) -> p t w", t=2),
                        lam128[:], pview(r_t, pl), op0=OP.mult, op1=OP.add)
            nc.vector.tensor_copy(p_t, r_t)
            nc.scalar.copy(p_b, p_t)
            nc.vector.memset(x_t, 0.0)

        # gpsimd: SBUF-only work (coil forming). vector: all PSUM readers.

        def dft_stage(src, dirn, tag):
            """src interleaved FPR [128,1024] -> 2 psum tiles [128,512]."""
            ps = [T(pp, [128, 512], "ps") for m in range(2)]
            for t in range(2):
                for pl in range(2):
                    fmt = fm[dirn * 4 + pl * 2 + t]
                    for m in range(2):
                        lt = src[:, t * 512 + pl * 256 + m * 128:
                                 t * 512 + pl * 256 + m * 128 + 128]
                        nc.tensor.matmul(ps[m], lt, fmt,
                                         start=(t == 0 and pl == 0),
                                         stop=(t == 1 and pl == 1))
            return ps

        # ---------------- CG loop ----------------
        for it in range(n_cg):
            # rr = <r,r> early (overlaps AtA): square on gpsimd, row-reduce
            # on vector into the packed partials tile.
            rscr = T(wq, [128, 1024], "rscr")
            nc.gpsimd.tensor_tensor(rscr, r_t, r_t, op=OP.mult)
            nc.vector.tensor_reduce(dotp[:, 3:4], rscr, axis=AX.X, op=OP.add)

            # --- Ap = AtA(p): 3 groups of 4 coils ---
            for g in range(3):
                prodA = T(prodp, [128, 4096], "prodA", BF)
                prodB = T(prodp, [128, 4096], "prodB", BF)
                for ci2 in range(2):
                    # two coils software-pipelined: drains of one coil hide
                    # under the other coil's matmuls (no PE head-of-line)
                    cis = (ci2 * 2, ci2 * 2 + 1)
                    coils, flds = {}, {}
                    for ci in cis:
                        c = g * 4 + ci
                        coil = T(wk, [128, 1024], "coil", BF)
                        fs0 = T(wk, [128, 512], "fs0", BF)
                        fs1 = T(wk, [128, 512], "fs1", BF)
                        av = pview(csm[c], 0)
                        bv = pview(csm[c], 1)
                        uv = pview(p_b, 0)
                        vv = pview(p_b, 1)
                        f0v = fs0.rearrange("p (t w) -> p t w", t=2)
                        f1v = fs1.rearrange("p (t w) -> p t w", t=2)
                        first = (g == 0 and ci2 == 0)
                        gp = (nc.gpsimd if (gp_form and not first)
                              else nc.vector)
                        gp.tensor_tensor(f0v, av, uv, op=OP.mult)
                        gp.tensor_tensor(f1v, bv, vv, op=OP.mult)
                        gp.tensor_tensor(pview(coil, 0), f0v, f1v,
                                         op=OP.subtract)
                        gp.tensor_tensor(f0v, av, vv, op=OP.mult)
                        gp.tensor_tensor(f1v, bv, uv, op=OP.mult)
                        gp.tensor_tensor(pview(coil, 1), f0v, f1v, op=OP.add)
                        coils[ci] = coil
                    # stage 1
                    for ci in cis:
                        ps1 = dft_stage(coils[ci], 0, "s1")
                        Tt = T(wk, [128, 1024], "Tt", BF)
                        for m in range(2):
                            nc.scalar.copy(Tt[:, m * 512:(m + 1) * 512],
                                           ps1[m])
                        flds[ci] = Tt
                    # stage 2 (+mask)
                    for ci in cis:
                        ps2 = dft_stage(flds[ci], 0, "s2")
                        Zt = T(wk, [128, 1024], "Zt", BF)
                        for m in range(2):
                            nc.vector.tensor_tensor(
                                Zt[:, m * 512:(m + 1) * 512], ps2[m],
                                mask2[:, m * 512:(m + 1) * 512], op=OP.mult)
                        flds[ci] = Zt
                    # stage 3
                    for ci in cis:
                        ps3 = dft_stage(flds[ci], 1, "s3")
                        Ut = T(wk, [128, 1024], "Ut", BF)
                        for m in range(2):
                            nc.scalar.copy(Ut[:, m * 512:(m + 1) * 512],
                                           ps3[m])
                        flds[ci] = Ut
                    # stage 4 + products
                    for ci in cis:
                        c = g * 4 + ci
                        ps4 = dft_stage(flds[ci], 1, "s4")
                        for m in range(2):
                            base = ci * 1024 + m * 512
                            w4b = T(wk, [128, 512], f"w4b{m}", BF)
                            nc.scalar.copy(w4b, ps4[m])
                            nc.vector.tensor_tensor(
                                prodA[:, base:base + 512],
                                csm[c][:, m * 512:(m + 1) * 512],
                                w4b, op=OP.mult)
                            nc.vector.tensor_tensor(
                                prodB[:, base:base + 512],
                                csw[c][:, m * 512:(m + 1) * 512],
                                w4b, op=OP.mult)
                # tree-reduce 4 coil slots -> accs[g][pl], then fold into
                # ap incrementally (overlaps the next group's matmuls)
                for pl, (prod, te) in enumerate(
                        ((prodA, nc.vector), (prodB, nc.vector))):
                    sfx = "v" if pl == 0 else "g"
                    q1 = T(wq, [128, 1024], f"q1{sfx}", BF)
                    q2 = T(wq, [128, 1024], f"q2{sfx}", BF)
                    te.tensor_tensor(q1, prod[:, 0:1024], prod[:, 2048:3072],
                                     op=OP.add)
                    te.tensor_tensor(q2, prod[:, 1024:2048], prod[:, 3072:4096],
                                     op=OP.add)
                    te.tensor_tensor(q1, q1, q2, op=OP.add)
                    q1v = q1.rearrange("p (t h w) -> p t h w", t=2, h=2)
                    te.tensor_tensor(
                        accs[g][pl].rearrange("p (t w) -> p t w", t=2),
                        q1v[:, :, 0], q1v[:, :, 1], op=OP.add)
                    apv = pview(ap_t, pl)
                    accv = accs[g][pl].rearrange("p (t w) -> p t w", t=2)
                    if g == 0:
                        te.scalar_tensor_tensor(
                            apv, pview(p_t, pl), lam128[:], accv,
                            op0=OP.mult, op1=OP.add)
                    else:
                        te.tensor_tensor(apv, apv, accv, op=OP.add)

            # dots: ApAp square on gpsimd (parallel), pAp/rAp on vector
            dsc3 = T(wq, [128, 1024], "dscg")
            nc.gpsimd.tensor_tensor(dsc3, ap_t, ap_t, op=OP.mult)
            for k, lhs in enumerate((p_t, r_t)):
                dsc = T(wq, [128, 1024], "dscr")
                nc.vector.tensor_tensor(dsc, lhs, ap_t, op=OP.mult)
                nc.vector.tensor_reduce(dotp[:, k:k + 1], dsc,
                                        axis=AX.X, op=OP.add)
            nc.vector.tensor_reduce(dotp[:, 2:3], dsc3, axis=AX.X, op=OP.add)

            # ---- partition-sum via ones-vector matmul, then a 16B
            # allreduce and a single broadcast DMA back ----
            psd = T(pp, [128, 512], "ps")
            nc.tensor.matmul(psd[0:1, 0:4], ones128, dotp,
                             start=True, stop=True)
            g13 = T(sc, [1, 4], "ccg13")
            nc.scalar.copy(g13, psd[0:1, 0:4])
            nc.gpsimd.dma_start(out=ccin[it][:], in_=g13)
            if use_cc:
                nc.gpsimd.collective_compute(
                    "AllReduce", OP.add, replica_groups=group,
                    ins=[ccin[it][:]], outs=[ccout[it][:]])
                src_cc = ccout[it]
            else:
                src_cc = ccin[it]
            scal4 = T(sc, [128, 4], "scal4")
            nc.gpsimd.dma_start(out=scal4,
                                in_=src_cc[:].to_broadcast([128, 4]))

            # alpha = rr/pAp ; rTrNew = rr - 2a*rAp + a^2*ApAp ; beta
            rec = T(sc, [128, 1], "rec")
            nc.vector.reciprocal(rec, scal4[:, 0:1])
            alpha = T(sc, [128, 1], "alpha")
            nc.vector.tensor_tensor(alpha, scal4[:, 3:4], rec, op=OP.mult)
            s2 = T(sc, [128, 1], "s2")
            nc.vector.tensor_tensor(s2, alpha, scal4[:, 1:2], op=OP.mult)
            a2 = T(sc, [128, 1], "a2")
            nc.vector.tensor_tensor(a2, alpha, alpha, op=OP.mult)
            s1 = T(sc, [128, 1], "s1")
            nc.vector.tensor_tensor(s1, a2, scal4[:, 2:3], op=OP.mult)
            tmp = T(sc, [128, 1], "tmp")
            nc.vector.scalar_tensor_tensor(tmp, s2, -2.0, scal4[:, 3:4],
                                           op0=OP.mult, op1=OP.add)
            rtrnew = T(sc, [128, 1], "rtrnew")
            nc.vector.tensor_tensor(rtrnew, tmp, s1, op=OP.add)
            nalpha = T(sc, [128, 1], "nalpha")
            nc.vector.tensor_scalar_mul(nalpha, alpha, -1.0)

            # updates on vector: x (reads old p), r, then p
            nc.vector.scalar_tensor_tensor(x_t, p_t, alpha[:], x_t,
                                           op0=OP.mult, op1=OP.add)
            nc.vector.scalar_tensor_tensor(r_t, ap_t, nalpha[:], r_t,
                                           op0=OP.mult, op1=OP.add)
            if it < n_cg - 1:
                rec2 = T(sc, [128, 1], "rec2")
                nc.vector.reciprocal(rec2, scal4[:, 3:4])
                beta = T(sc, [128, 1], "beta")
                nc.vector.tensor_tensor(beta, rtrnew, rec2, op=OP.mult)
                nc.vector.scalar_tensor_tensor(p_t, p_t, beta[:], r_t,
                                               op0=OP.mult, op1=OP.add)
                nc.scalar.copy(p_b, p_t)

        # ---- output ----
        for pl in range(2):
            nc.sync.dma_start(
                out=dout[pl].rearrange("(t p w) -> p t w", t=2, p=128),
                in_=pview(x_t, pl))

    if evsem:
        import bass_rust as _bass_rust
        _bass_rust.generate_event_semaphores(nc)
        mybir.codegen_inst_isa_subclasses(nc)
    return nc


# ------------------------------------------------------------------ runner

_CACHE = {}


def _get_nc(key=(N_CG, NCOIL, N_CORES, True)):
    if key not in _CACHE:
        _CACHE[key] = build_nc(*key)
    return _CACHE[key]


def make_in_maps(inputs):
    shared = _prep_shared(
        inputs["w1"], inputs["b1"], inputs["w2"], inputs["b2"], inputs["w3"],
        inputs["b3"], inputs["w4"], inputs["b4"], inputs["w5"], inputs["b5"],
        inputs["lam"])
    in_maps = []
    for b in range(N_CORES):
        m = dict(shared)
        m.update(_prep_core(
            np.asarray(inputs["atb"][b], np.float32),
            np.asarray(inputs["csm_real"][b], np.float32),
            np.asarray(inputs["csm_imag"][b], np.float32),
            np.asarray(inputs["mask"][b], np.float32)))
        in_maps.append(m)
    return in_maps


def run(inputs, trace=False, **kw):
    nc = _get_nc()
    in_maps = make_in_maps(inputs)
    res = run_bass_kernel_spmd(nc, in_maps, core_ids=list(range(N_CORES)),
                               trace=trace, **kw)
    out = np.stack([np.asarray(r["out"]).reshape(2, 256, 256)
                    for r in res.results]).astype(np.float32)
    return out, res


def kernel(**inputs):
    out, _ = run(inputs, trace=False)
    return out

